# revision 7
# baseline (speedup 1.0000x reference)
"""Distributed 2-layer GCN on 8 NeuronCores (Trainium2, Bass/Tile).

Strategy (graph-partition parallelism):
  - Rows (owned nodes) are degree-sorted and dealt round-robin to the 8
    cores in 128-row blocks so every core gets an identical static
    schedule (SPMD: one traced program).
  - Both GCN layers are computed "aggregate-first":
        out = ((A @ (x*deg)) * deg) @ W + b
    which is algebraically identical to the reference
    (D^-1/2 A D^-1/2 x W + b) because the row/col scalings and the dense
    projection commute with the sparse aggregation.
  - The sparse aggregation runs as: bulk int16 dma_gather (4 parallel
    SWDGE queues) of 256B node rows from a DRAM table, then a one-hot
    "scatter matmul" on the PE accumulating each 128-edge chunk into the
    block's PSUM tile.  One-hots are built on the DVE via iota==rowid.
  - The gather universe (55040 / 50176 rows) exceeds int16, so each
    block's edges are split into two overlapping 32768-row windows.
  - Layer-2 inputs (y2 = relu(out1)*deg) are exchanged with a DRAM
    AllGather across the 8 cores, then layer 2 repeats the same pipeline
    reading from the gathered table.
"""

import numpy as np
import ml_dtypes

N_LOCAL = 55000
N_OWN = 50000
N_EDGES = 800000
C = 128          # in/hidden channels
C2 = 64          # out channels
NC = 8
P = 128
GROUP = NC * P                    # 1024 rows dealt per block index
NB = (N_OWN + GROUP - 1) // GROUP  # 49 blocks per core
SLOTS = NB * P                    # 6272 row slots per core
V1 = 55040                        # layer-1 gather table rows (padded)
W16 = 32768                       # int16 window width
BASE1 = V1 - W16                  # 22272
V2 = NC * SLOTS                   # 50176 layer-2 table rows
BASE2 = V2 - W16                  # 17408
BF16 = ml_dtypes.bfloat16

_PROGRAM_CACHE = {}


# ----------------------------------------------------------------------
# Host-side schedule construction (pure numpy; edges are inputs)
# ----------------------------------------------------------------------

def _build_schedule(edge_row, edge_col, deg):
    """Returns per-core index/one-hot tensors + static chunk schedule."""
    er = edge_row.astype(np.int64)
    ec = edge_col.astype(np.int64)
    keep = er < N_OWN
    er, ec = er[keep], ec[keep]

    deg_cnt = np.bincount(er, minlength=N_OWN)
    order = np.argsort(-deg_cnt, kind="stable").astype(np.int64)  # rank -> row
    inv_order = np.empty(N_OWN, np.int64)
    inv_order[order] = np.arange(N_OWN)

    # rank -> (core, block, partition);  slot s of rank r:
    #   g = r // 1024, lane = (r % 1024) // 128, p = r % 128
    rank_of = inv_order  # row -> rank
    # per-edge destination
    e_rank = rank_of[er]
    e_g = e_rank // GROUP
    e_lane = (e_rank % GROUP) // P
    e_p = e_rank % P

    # layer-2 source position of a col (only cols < N_OWN)
    def pos2_of(col):
        r = rank_of[col]
        return (r % GROUP) // P * SLOTS + (r // GROUP) * P + (r % P)

    # ---- per (core, block, window) edge lists --------------------------
    # layer 1: idx = col (table1 position), window by col
    # layer 2: idx = pos2(col), dropped if col >= N_OWN
    lists1 = [[[[], []] for _ in range(NB)] for _ in range(NC)]
    lists2 = [[[[], []] for _ in range(NB)] for _ in range(NC)]
    l2_valid = ec < N_OWN
    e_pos2 = np.zeros(len(ec), np.int64)
    e_pos2[l2_valid] = pos2_of(ec[l2_valid])
    for i in range(len(er)):
        k, b, p = e_lane[i], e_g[i], e_p[i]
        c1 = ec[i]
        w1 = 0 if c1 < W16 else 1
        lists1[k][b][w1].append((c1 - (BASE1 if w1 else 0), p))
        if l2_valid[i]:
            c2 = e_pos2[i]
            w2 = 0 if c2 < W16 else 1
            lists2[k][b][w2].append((c2 - (BASE2 if w2 else 0), p))

    def pack(lists):
        # static chunk counts (max over cores)
        K = np.zeros((NB, 2), np.int64)
        for b in range(NB):
            for w in range(2):
                n = max(len(lists[k][b][w]) for k in range(NC))
                K[b, w] = (n + P - 1) // P
            if K[b, 0] + K[b, 1] == 0:
                K[b, 0] = 1  # ensure PSUM init via all-dead chunk
        tot_chunks = int(K.sum())
        tot_idx = tot_chunks * P
        idx16 = np.zeros((NC, 128, tot_idx // 16), np.int16)
        rowloc = np.full((NC, 128, tot_chunks), 128.0, np.float32)
        off_chunk = 0
        offs = []
        for b in range(NB):
            for w in range(2):
                kc = int(K[b, w])
                offs.append((b, w, off_chunk, kc))
                if kc == 0:
                    continue
                n_idx = kc * P
                for k in range(NC):
                    lst = lists[k][b][w]
                    loc = np.zeros(n_idx, np.int64)
                    rl = np.full(n_idx, 128.0, np.float32)
                    if lst:
                        a = np.asarray(lst, np.int64)
                        loc[: len(a)] = a[:, 0]
                        rl[: len(a)] = a[:, 1]
                    # idx j -> partition j%128, stripe j//128; chunk c = js [c*128,(c+1)*128)
                    rowloc[k, :, off_chunk : off_chunk + kc] = (
                        rl.reshape(kc, P).T
                    )
                    wrapped = loc.reshape(n_idx // 16, 16).T.astype(np.int16)  # [16, S16]
                    idx16[k, :, off_chunk * 8 : (off_chunk + kc) * 8] = np.tile(
                        wrapped, (8, 1)
                    )
                off_chunk += kc
        return K, idx16, rowloc, offs

    K1, idx16_1, rowloc1, offs1 = pack(lists1)
    K2, idx16_2, rowloc2, offs2 = pack(lists2)

    # per-core owned-row deg (0 for pad slots), [128, NB]
    degO = np.zeros((NC, 128, NB), np.float32)
    row_of_slot = np.full((NC, SLOTS), -1, np.int64)
    for k in range(NC):
        for b in range(NB):
            ranks = b * GROUP + k * P + np.arange(P)
            valid = ranks < N_OWN
            rows = np.where(valid, order[np.minimum(ranks, N_OWN - 1)], -1)
            row_of_slot[k, b * P : (b + 1) * P] = rows
            degO[k, valid, b] = deg[rows[valid]]
    return dict(
        K1=K1, idx16_1=idx16_1, rowloc1=rowloc1, offs1=offs1,
        K2=K2, idx16_2=idx16_2, rowloc2=rowloc2, offs2=offs2,
        degO=degO, row_of_slot=row_of_slot, order=order,
    )


# ----------------------------------------------------------------------
# Device program
# ----------------------------------------------------------------------

def _build_program(K1, offs1, K2, offs2):
    import concourse.bass as bass
    import concourse.bacc as bacc
    import concourse.tile as tile
    import concourse.mybir as mybir

    S16_1 = int(K1.sum()) * 8
    NCH1 = int(K1.sum())
    S16_2 = int(K2.sum()) * 8
    NCH2 = int(K2.sum())
    KMAX = int(max(K1.max(), K2.max()))

    nc = bacc.Bacc("TRN2", target_bir_lowering=False, debug=False,
                   num_devices=NC, num_swdge_queues=4)
    dt = mybir.dt
    table1 = nc.dram_tensor("table1", [V1, C], dt.bfloat16, kind="ExternalInput")
    idx1_d = nc.dram_tensor("idx1", [128, S16_1], dt.int16, kind="ExternalInput")
    rowloc1_d = nc.dram_tensor("rowloc1", [128, NCH1], dt.float32, kind="ExternalInput")
    idx2_d = nc.dram_tensor("idx2", [128, S16_2], dt.int16, kind="ExternalInput")
    rowloc2_d = nc.dram_tensor("rowloc2", [128, NCH2], dt.float32, kind="ExternalInput")
    degO_d = nc.dram_tensor("degO", [128, NB], dt.float32, kind="ExternalInput")
    w1_d = nc.dram_tensor("w1", [C, C], dt.bfloat16, kind="ExternalInput")
    w2_d = nc.dram_tensor("w2", [C, C2], dt.bfloat16, kind="ExternalInput")
    b1_d = nc.dram_tensor("b1", [C, 1], dt.float32, kind="ExternalInput")
    b2_d = nc.dram_tensor("b2", [C2, 1], dt.float32, kind="ExternalInput")
    ident_d = nc.dram_tensor("ident", [128, 128], dt.bfloat16, kind="ExternalInput")
    iota_d = nc.dram_tensor("iota", [128, 128], dt.bfloat16, kind="ExternalInput")
    out_d = nc.dram_tensor("outT", [C2, SLOTS], dt.float32, kind="ExternalOutput")

    qrr = [0]

    def next_q():
        q = qrr[0]
        qrr[0] = (q + 1) % 4
        return q

    with tile.TileContext(nc) as tc:
        with (
            tc.tile_pool(name="const", bufs=1) as cpool,
            tc.tile_pool(name="gather", bufs=4) as gpool,
            tc.tile_pool(name="onehot", bufs=8) as opool,
            tc.tile_pool(name="agg", bufs=4, space="PSUM") as agg_pool,
            tc.tile_pool(name="trp", bufs=2, space="PSUM") as tr_pool,
            tc.tile_pool(name="proj", bufs=2, space="PSUM") as proj_pool,
            tc.tile_pool(name="dram", bufs=1, space="DRAM") as dpool,
        ):
            idx1_sb = cpool.tile([128, S16_1], dt.int16)
            nc.sync.dma_start(out=idx1_sb[:], in_=idx1_d[:])
            rowloc1_sb = cpool.tile([128, NCH1], dt.float32)
            nc.sync.dma_start(out=rowloc1_sb[:], in_=rowloc1_d[:])
            idx2_sb = cpool.tile([128, S16_2], dt.int16)
            nc.sync.dma_start(out=idx2_sb[:], in_=idx2_d[:])
            rowloc2_sb = cpool.tile([128, NCH2], dt.float32)
            nc.sync.dma_start(out=rowloc2_sb[:], in_=rowloc2_d[:])
            degO_sb = cpool.tile([128, NB], dt.float32)
            nc.sync.dma_start(out=degO_sb[:], in_=degO_d[:])
            w1_sb = cpool.tile([C, C], dt.bfloat16)
            nc.sync.dma_start(out=w1_sb[:], in_=w1_d[:])
            w2_sb = cpool.tile([C, C2], dt.bfloat16)
            nc.sync.dma_start(out=w2_sb[:], in_=w2_d[:])
            b1_sb = cpool.tile([C, 1], dt.float32)
            nc.sync.dma_start(out=b1_sb[:], in_=b1_d[:])
            b2_sb = cpool.tile([C2, 1], dt.float32)
            nc.sync.dma_start(out=b2_sb[:], in_=b2_d[:])
            ident_sb = cpool.tile([128, 128], dt.bfloat16)
            nc.sync.dma_start(out=ident_sb[:], in_=ident_d[:])
            iota_sb = cpool.tile([128, 128], dt.bfloat16)
            nc.sync.dma_start(out=iota_sb[:], in_=iota_d[:])

            y2loc = dpool.tile([SLOTS, C], dt.bfloat16)
            y2full = dpool.tile([V2, C], dt.bfloat16)

            def aggregate(block, Ktab, offs_map, idx_sb, rowloc_sb, srcs):
                """One 128-row block: gathers + one-hot scatter matmuls."""
                agg = agg_pool.tile([128, C], dt.float32, tag="agg")
                chunks = [(w, off, kc) for (b, w, off, kc) in offs_map[block] if kc > 0]
                total = sum(kc for (_, _, kc) in chunks)
                done = 0
                for (w, off, kc) in chunks:
                    g = gpool.tile([128, KMAX, C], dt.bfloat16, tag="g")
                    n_idx = kc * P
                    nc.gpsimd.dma_gather(
                        out_ap=g[:, 0:kc, :],
                        in_ap=srcs[w],
                        idxs_ap=idx_sb[:, off * 8 : (off + kc) * 8],
                        num_idxs=n_idx, num_idxs_reg=n_idx,
                        elem_size=C, queue_num=next_q(),
                        single_packet=(n_idx <= 1024),
                    )
                    for c in range(kc):
                        S = opool.tile([128, 128], dt.bfloat16, tag="S")
                        nc.vector.tensor_scalar(
                            out=S[:], in0=iota_sb[:],
                            scalar1=rowloc_sb[:, off + c : off + c + 1],
                            scalar2=None, op0=mybir.AluOpType.is_equal,
                        )
                        nc.tensor.matmul(
                            agg[:], lhsT=S[:], rhs=g[:, c, :],
                            start=(done == 0), stop=(done == total - 1),
                        )
                        done += 1
                return agg

            # organize offs per block
            offs1_map = [[] for _ in range(NB)]
            for t in offs1:
                offs1_map[t[0]].append(t)
            offs2_map = [[] for _ in range(NB)]
            for t in offs2:
                offs2_map[t[0]].append(t)

            src1 = [table1[0:W16, :], table1[BASE1:, :]]
            src2 = [y2full[0:W16, :], y2full[BASE2:, :]]

            # ---------------- layer 1 ----------------
            T1 = cpool.tile([128, SLOTS], dt.bfloat16)
            for b in range(NB):
                agg = aggregate(b, K1, offs1_map, idx1_sb, rowloc1_sb, src1)
                nc.vector.tensor_scalar(
                    out=T1[:, b * P : (b + 1) * P], in0=agg[:],
                    scalar1=degO_sb[:, b : b + 1], scalar2=None,
                    op0=mybir.AluOpType.mult,
                )
            # transpose blocks: T1 (node-major) -> TT1 (feature-major)
            TT1 = cpool.tile([128, SLOTS], dt.bfloat16)
            for b in range(NB):
                trp = tr_pool.tile([128, 128], dt.bfloat16, tag="tr")
                nc.tensor.transpose(trp[:], T1[:, b * P : (b + 1) * P], ident_sb[:])
                nc.scalar.copy(TT1[:, b * P : (b + 1) * P], trp[:])
            # projection + relu + bias
            X2T = cpool.tile([128, SLOTS], dt.bfloat16)
            NPROJ = 512
            for j in range(0, SLOTS, NPROJ):
                n = min(NPROJ, SLOTS - j)
                pp = proj_pool.tile([128, NPROJ], dt.float32, tag="proj")
                nc.tensor.matmul(pp[:, 0:n], lhsT=w1_sb[:], rhs=TT1[:, j : j + n],
                                 start=True, stop=True)
                nc.scalar.activation(
                    X2T[:, j : j + n], pp[:, 0:n],
                    mybir.ActivationFunctionType.Relu, bias=b1_sb[:, 0:1],
                )
            # back-transpose + *deg -> y2 (node-major, bf16) -> DRAM
            y2sb = cpool.tile([128, SLOTS], dt.bfloat16)
            for b in range(NB):
                trp = tr_pool.tile([128, 128], dt.bfloat16, tag="tr")
                nc.tensor.transpose(trp[:], X2T[:, b * P : (b + 1) * P], ident_sb[:])
                nc.vector.tensor_scalar(
                    out=y2sb[:, b * P : (b + 1) * P], in0=trp[:],
                    scalar1=degO_sb[:, b : b + 1], scalar2=None,
                    op0=mybir.AluOpType.mult,
                )
            nc.sync.dma_start(
                out=y2loc[:].rearrange("(b p) c -> p b c", p=128),
                in_=y2sb[:].rearrange("p (b c) -> p b c", b=NB),
            )
            # ---------------- exchange ----------------
            nc.gpsimd.collective_compute(
                "AllGather", mybir.AluOpType.bypass,
                replica_groups=[list(range(NC))],
                ins=[y2loc[:].opt()], outs=[y2full[:].opt()],
            )
            # ---------------- layer 2 ----------------
            T2 = cpool.tile([128, SLOTS], dt.bfloat16)
            for b in range(NB):
                agg = aggregate(b, K2, offs2_map, idx2_sb, rowloc2_sb, src2)
                nc.vector.tensor_scalar(
                    out=T2[:, b * P : (b + 1) * P], in0=agg[:],
                    scalar1=degO_sb[:, b : b + 1], scalar2=None,
                    op0=mybir.AluOpType.mult,
                )
            TT2 = cpool.tile([128, SLOTS], dt.bfloat16)
            for b in range(NB):
                trp = tr_pool.tile([128, 128], dt.bfloat16, tag="tr")
                nc.tensor.transpose(trp[:], T2[:, b * P : (b + 1) * P], ident_sb[:])
                nc.scalar.copy(TT2[:, b * P : (b + 1) * P], trp[:])
            OUT = cpool.tile([C2, SLOTS], dt.float32)
            for j in range(0, SLOTS, NPROJ):
                n = min(NPROJ, SLOTS - j)
                pp = proj_pool.tile([C2, NPROJ], dt.float32, tag="proj")
                nc.tensor.matmul(pp[:, 0:n], lhsT=w2_sb[:], rhs=TT2[:, j : j + n],
                                 start=True, stop=True)
                nc.scalar.activation(
                    OUT[:, j : j + n], pp[:, 0:n],
                    mybir.ActivationFunctionType.Identity, bias=b2_sb[:, 0:1],
                )
            nc.sync.dma_start(out=out_d[:], in_=OUT[:])
    nc.compile()
    return nc


# ----------------------------------------------------------------------
# Entry point
# ----------------------------------------------------------------------

def kernel(x, deg_inv_sqrt, w1, b1, w2, b2, edge_row, edge_col, num_owned):
    from concourse import bass_utils

    x = np.asarray(x, np.float32)
    deg = np.asarray(deg_inv_sqrt, np.float32)
    sched = _build_schedule(np.asarray(edge_row), np.asarray(edge_col), deg)

    key = (
        sched["K1"].tobytes(), sched["K2"].tobytes(),
    )
    if key not in _PROGRAM_CACHE:
        _PROGRAM_CACHE[key] = _build_program(
            sched["K1"], sched["offs1"], sched["K2"], sched["offs2"]
        )
    nc = _PROGRAM_CACHE[key]

    table1 = np.zeros((V1, C), BF16)
    table1[:N_LOCAL] = (x * deg[:, None]).astype(BF16)
    iota_np = np.tile(np.arange(128, dtype=BF16)[None, :], (128, 1))
    ident_np = np.eye(128, dtype=BF16)
    w1_b = np.asarray(w1, np.float32).astype(BF16)
    w2_b = np.asarray(w2, np.float32).astype(BF16)
    b1_c = np.asarray(b1, np.float32).reshape(C, 1)
    b2_c = np.asarray(b2, np.float32).reshape(C2, 1)

    in_maps = []
    for k in range(NC):
        in_maps.append({
            "table1": table1,
            "idx1": sched["idx16_1"][k],
            "rowloc1": sched["rowloc1"][k],
            "idx2": sched["idx16_2"][k],
            "rowloc2": sched["rowloc2"][k],
            "degO": sched["degO"][k],
            "w1": w1_b, "w2": w2_b, "b1": b1_c, "b2": b2_c,
            "ident": ident_np, "iota": iota_np,
        })
    res = bass_utils.run_bass_kernel_spmd(nc, in_maps, core_ids=list(range(NC)))

    out = np.zeros((N_OWN, C2), np.float32)
    for k in range(NC):
        got = res.results[k]["outT"]  # [C2, SLOTS]
        rows = sched["row_of_slot"][k]
        valid = rows >= 0
        out[rows[valid]] = got[:, valid].T
    return out


# revision 8
# speedup vs baseline: 1.6248x; 1.6248x over previous
"""Distributed 2-layer GCN on 8 NeuronCores (Trainium2, Bass/Tile).

Strategy (graph-partition parallelism):
  - Rows (owned nodes) are degree-sorted and dealt round-robin to the 8
    cores in 128-row blocks so every core gets an identical static
    schedule (SPMD: one traced program).
  - Both GCN layers are computed "aggregate-first":
        out = ((A @ (x*deg)) * deg) @ W + b
    which is algebraically identical to the reference
    (D^-1/2 A D^-1/2 x W + b) because the row/col scalings and the dense
    projection commute with the sparse aggregation.
  - The sparse aggregation runs as: bulk int16 dma_gather (4 parallel
    SWDGE queues) of 256B node rows from a DRAM table, then a one-hot
    "scatter matmul" on the PE accumulating each 128-edge chunk into the
    block's PSUM tile.  One-hots are built on the DVE via iota==rowid.
  - The gather universe (55040 / 50176 rows) exceeds int16, so each
    block's edges are split into two overlapping 32768-row windows.
  - Layer-2 inputs (y2 = relu(out1)*deg) are exchanged with a DRAM
    AllGather across the 8 cores, then layer 2 repeats the same pipeline
    reading from the gathered table.
"""

import numpy as np
import ml_dtypes

N_LOCAL = 55000
N_OWN = 50000
N_EDGES = 800000
C = 128          # in/hidden channels
C2 = 64          # out channels
NC = 8
P = 128
GROUP = NC * P                    # 1024 rows dealt per block index
NB = (N_OWN + GROUP - 1) // GROUP  # 49 blocks per core
SLOTS = NB * P                    # 6272 row slots per core
V1 = 55040                        # layer-1 gather table rows (padded)
W16 = 32768                       # int16 window width
BASE1 = V1 - W16                  # 22272
V2 = NC * SLOTS                   # 50176 layer-2 table rows
BASE2 = V2 - W16                  # 17408
BF16 = ml_dtypes.bfloat16

_PROGRAM_CACHE = {}


# ----------------------------------------------------------------------
# Host-side schedule construction (pure numpy; edges are inputs)
# ----------------------------------------------------------------------

def _build_schedule(edge_row, edge_col, deg):
    """Returns per-core index/one-hot tensors + static chunk schedule."""
    er = edge_row.astype(np.int64)
    ec = edge_col.astype(np.int64)
    keep = er < N_OWN
    er, ec = er[keep], ec[keep]

    deg_cnt = np.bincount(er, minlength=N_OWN)
    order = np.argsort(-deg_cnt, kind="stable").astype(np.int64)  # rank -> row
    inv_order = np.empty(N_OWN, np.int64)
    inv_order[order] = np.arange(N_OWN)

    # rank -> (core, block, partition);  slot s of rank r:
    #   g = r // 1024, lane = (r % 1024) // 128, p = r % 128
    rank_of = inv_order  # row -> rank
    # per-edge destination
    e_rank = rank_of[er]
    e_g = e_rank // GROUP
    e_lane = (e_rank % GROUP) // P
    e_p = e_rank % P

    # layer-2 source position of a col (only cols < N_OWN)
    def pos2_of(col):
        r = rank_of[col]
        return (r % GROUP) // P * SLOTS + (r // GROUP) * P + (r % P)

    # ---- per (core, block, window) edge lists --------------------------
    # layer 1: idx = col (table1 position), window by col
    # layer 2: idx = pos2(col), dropped if col >= N_OWN
    lists1 = [[[[], []] for _ in range(NB)] for _ in range(NC)]
    lists2 = [[[[], []] for _ in range(NB)] for _ in range(NC)]
    l2_valid = ec < N_OWN
    e_pos2 = np.zeros(len(ec), np.int64)
    e_pos2[l2_valid] = pos2_of(ec[l2_valid])
    for i in range(len(er)):
        k, b, p = e_lane[i], e_g[i], e_p[i]
        c1 = ec[i]
        w1 = 0 if c1 < W16 else 1
        lists1[k][b][w1].append((c1 - (BASE1 if w1 else 0), p))
        if l2_valid[i]:
            c2 = e_pos2[i]
            w2 = 0 if c2 < W16 else 1
            lists2[k][b][w2].append((c2 - (BASE2 if w2 else 0), p))

    def pack(lists):
        # static chunk counts (max over cores)
        K = np.zeros((NB, 2), np.int64)
        for b in range(NB):
            for w in range(2):
                n = max(len(lists[k][b][w]) for k in range(NC))
                K[b, w] = (n + P - 1) // P
            if K[b, 0] + K[b, 1] == 0:
                K[b, 0] = 1  # ensure PSUM init via all-dead chunk
        tot_chunks = int(K.sum())
        tot_idx = tot_chunks * P
        idx16 = np.zeros((NC, 128, tot_idx // 16), np.int16)
        rowloc = np.full((NC, 128, tot_chunks), 128.0, BF16)
        off_chunk = 0
        offs = []
        for b in range(NB):
            for w in range(2):
                kc = int(K[b, w])
                offs.append((b, w, off_chunk, kc))
                if kc == 0:
                    continue
                n_idx = kc * P
                for k in range(NC):
                    lst = lists[k][b][w]
                    loc = np.zeros(n_idx, np.int64)
                    rl = np.full(n_idx, 128.0, np.float32)
                    if lst:
                        a = np.asarray(lst, np.int64)
                        loc[: len(a)] = a[:, 0]
                        rl[: len(a)] = a[:, 1]
                    # idx j -> partition j%128, stripe j//128; chunk c = js [c*128,(c+1)*128)
                    rowloc[k, :, off_chunk : off_chunk + kc] = (
                        rl.reshape(kc, P).T
                    )
                    wrapped = loc.reshape(n_idx // 16, 16).T.astype(np.int16)  # [16, S16]
                    idx16[k, :, off_chunk * 8 : (off_chunk + kc) * 8] = np.tile(
                        wrapped, (8, 1)
                    )
                off_chunk += kc
        return K, idx16, rowloc, offs

    K1, idx16_1, rowloc1, offs1 = pack(lists1)
    K2, idx16_2, rowloc2, offs2 = pack(lists2)

    # per-core owned-row deg (0 for pad slots), [128, NB]
    degO = np.zeros((NC, 128, NB), np.float32)
    row_of_slot = np.full((NC, SLOTS), -1, np.int64)
    for k in range(NC):
        for b in range(NB):
            ranks = b * GROUP + k * P + np.arange(P)
            valid = ranks < N_OWN
            rows = np.where(valid, order[np.minimum(ranks, N_OWN - 1)], -1)
            row_of_slot[k, b * P : (b + 1) * P] = rows
            degO[k, valid, b] = deg[rows[valid]]
    return dict(
        K1=K1, idx16_1=idx16_1, rowloc1=rowloc1, offs1=offs1,
        K2=K2, idx16_2=idx16_2, rowloc2=rowloc2, offs2=offs2,
        degO=degO, row_of_slot=row_of_slot, order=order,
    )


# ----------------------------------------------------------------------
# Device program
# ----------------------------------------------------------------------

def _build_program(K1, offs1, K2, offs2):
    import concourse.bass as bass
    import concourse.bacc as bacc
    import concourse.tile as tile
    import concourse.mybir as mybir

    S16_1 = int(K1.sum()) * 8
    NCH1 = int(K1.sum())
    S16_2 = int(K2.sum()) * 8
    NCH2 = int(K2.sum())
    KMAX = int(max(K1.max(), K2.max()))

    nc = bacc.Bacc("TRN2", target_bir_lowering=False, debug=False,
                   num_devices=NC, num_swdge_queues=4)
    dt = mybir.dt
    table1 = nc.dram_tensor("table1", [V1, C], dt.bfloat16, kind="ExternalInput")
    idx1_d = nc.dram_tensor("idx1", [128, S16_1], dt.int16, kind="ExternalInput")
    rowloc1_d = nc.dram_tensor("rowloc1", [128, NCH1], dt.bfloat16, kind="ExternalInput")
    idx2_d = nc.dram_tensor("idx2", [128, S16_2], dt.int16, kind="ExternalInput")
    rowloc2_d = nc.dram_tensor("rowloc2", [128, NCH2], dt.bfloat16, kind="ExternalInput")
    degO_d = nc.dram_tensor("degO", [128, NB], dt.float32, kind="ExternalInput")
    w1_d = nc.dram_tensor("w1", [C, C], dt.bfloat16, kind="ExternalInput")
    w2_d = nc.dram_tensor("w2", [C, C2], dt.bfloat16, kind="ExternalInput")
    b1_d = nc.dram_tensor("b1", [C, 1], dt.float32, kind="ExternalInput")
    b2_d = nc.dram_tensor("b2", [C2, 1], dt.float32, kind="ExternalInput")
    ident_d = nc.dram_tensor("ident", [128, 128], dt.bfloat16, kind="ExternalInput")
    iota_d = nc.dram_tensor("iota", [128, 128], dt.bfloat16, kind="ExternalInput")
    out_d = nc.dram_tensor("outT", [C2, SLOTS], dt.float32, kind="ExternalOutput")

    qrr = [0]

    def next_q():
        q = qrr[0]
        qrr[0] = (q + 1) % 4
        return q

    with tile.TileContext(nc) as tc:
        with (
            tc.tile_pool(name="const", bufs=1) as cpool,
            tc.tile_pool(name="gather", bufs=6) as gpool,
            tc.tile_pool(name="onehot", bufs=8) as opool,
            tc.tile_pool(name="agg", bufs=4, space="PSUM") as agg_pool,
            tc.tile_pool(name="trp", bufs=2, space="PSUM") as tr_pool,
            tc.tile_pool(name="proj", bufs=2, space="PSUM") as proj_pool,
            tc.tile_pool(name="dram", bufs=1, space="DRAM") as dpool,
        ):
            idx1_sb = cpool.tile([128, S16_1], dt.int16)
            nc.sync.dma_start(out=idx1_sb[:], in_=idx1_d[:])
            rowloc1_sb = cpool.tile([128, NCH1], dt.bfloat16)
            nc.sync.dma_start(out=rowloc1_sb[:], in_=rowloc1_d[:])
            idx2_sb = cpool.tile([128, S16_2], dt.int16)
            nc.sync.dma_start(out=idx2_sb[:], in_=idx2_d[:])
            rowloc2_sb = cpool.tile([128, NCH2], dt.bfloat16)
            nc.sync.dma_start(out=rowloc2_sb[:], in_=rowloc2_d[:])
            degO_sb = cpool.tile([128, NB], dt.float32)
            nc.sync.dma_start(out=degO_sb[:], in_=degO_d[:])
            w1_sb = cpool.tile([C, C], dt.bfloat16)
            nc.sync.dma_start(out=w1_sb[:], in_=w1_d[:])
            w2_sb = cpool.tile([C, C2], dt.bfloat16)
            nc.sync.dma_start(out=w2_sb[:], in_=w2_d[:])
            b1_sb = cpool.tile([C, 1], dt.float32)
            nc.sync.dma_start(out=b1_sb[:], in_=b1_d[:])
            b2_sb = cpool.tile([C2, 1], dt.float32)
            nc.sync.dma_start(out=b2_sb[:], in_=b2_d[:])
            ident_sb = cpool.tile([128, 128], dt.bfloat16)
            nc.sync.dma_start(out=ident_sb[:], in_=ident_d[:])
            iota_sb = cpool.tile([128, 128], dt.bfloat16)
            nc.sync.dma_start(out=iota_sb[:], in_=iota_d[:])

            y2loc = dpool.tile([SLOTS, C], dt.bfloat16)
            y2full = dpool.tile([V2, C], dt.bfloat16)

            def aggregate(block, Ktab, offs_map, idx_sb, rowloc_sb, srcs):
                """One 128-row block: gathers + one-hot scatter matmuls."""
                agg = agg_pool.tile([128, C], dt.float32, tag="agg")
                chunks = [(w, off, kc) for (b, w, off, kc) in offs_map[block] if kc > 0]
                total = sum(kc for (_, _, kc) in chunks)
                done = 0
                for (w, off, kc) in chunks:
                    g = gpool.tile([128, KMAX, C], dt.bfloat16, tag="g")
                    n_idx = kc * P
                    nc.gpsimd.dma_gather(
                        out_ap=g[:, 0:kc, :],
                        in_ap=srcs[w],
                        idxs_ap=idx_sb[:, off * 8 : (off + kc) * 8],
                        num_idxs=n_idx, num_idxs_reg=n_idx,
                        elem_size=C, queue_num=next_q(),
                        single_packet=(n_idx <= 1024),
                    )
                    for c in range(kc):
                        S = opool.tile([128, 128], dt.bfloat16, tag="S")
                        nc.vector.tensor_tensor(
                            out=S[:], in0=iota_sb[:],
                            in1=rowloc_sb[:, off + c : off + c + 1].to_broadcast([128, 128]),
                            op=mybir.AluOpType.is_equal,
                        )
                        nc.tensor.matmul(
                            agg[:], lhsT=S[:], rhs=g[:, c, :],
                            start=(done == 0), stop=(done == total - 1),
                        )
                        done += 1
                return agg

            # organize offs per block
            offs1_map = [[] for _ in range(NB)]
            for t in offs1:
                offs1_map[t[0]].append(t)
            offs2_map = [[] for _ in range(NB)]
            for t in offs2:
                offs2_map[t[0]].append(t)

            src1 = [table1[0:W16, :], table1[BASE1:, :]]
            src2 = [y2full[0:W16, :], y2full[BASE2:, :]]

            # ---------------- layer 1 ----------------
            T1 = cpool.tile([128, SLOTS], dt.bfloat16)
            for b in range(NB):
                agg = aggregate(b, K1, offs1_map, idx1_sb, rowloc1_sb, src1)
                nc.vector.tensor_scalar(
                    out=T1[:, b * P : (b + 1) * P], in0=agg[:],
                    scalar1=degO_sb[:, b : b + 1], scalar2=None,
                    op0=mybir.AluOpType.mult,
                )
            # transpose blocks: T1 (node-major) -> TT1 (feature-major)
            TT1 = cpool.tile([128, SLOTS], dt.bfloat16)
            for b in range(NB):
                trp = tr_pool.tile([128, 128], dt.bfloat16, tag="tr")
                nc.tensor.transpose(trp[:], T1[:, b * P : (b + 1) * P], ident_sb[:])
                nc.scalar.copy(TT1[:, b * P : (b + 1) * P], trp[:])
            # projection + relu + bias
            X2T = cpool.tile([128, SLOTS], dt.bfloat16)
            NPROJ = 512
            for j in range(0, SLOTS, NPROJ):
                n = min(NPROJ, SLOTS - j)
                pp = proj_pool.tile([128, NPROJ], dt.float32, tag="proj")
                nc.tensor.matmul(pp[:, 0:n], lhsT=w1_sb[:], rhs=TT1[:, j : j + n],
                                 start=True, stop=True)
                nc.scalar.activation(
                    X2T[:, j : j + n], pp[:, 0:n],
                    mybir.ActivationFunctionType.Relu, bias=b1_sb[:, 0:1],
                )
            # back-transpose + *deg -> y2 (node-major, bf16) -> DRAM
            y2sb = cpool.tile([128, SLOTS], dt.bfloat16)
            for b in range(NB):
                trp = tr_pool.tile([128, 128], dt.bfloat16, tag="tr")
                nc.tensor.transpose(trp[:], X2T[:, b * P : (b + 1) * P], ident_sb[:])
                nc.vector.tensor_scalar(
                    out=y2sb[:, b * P : (b + 1) * P], in0=trp[:],
                    scalar1=degO_sb[:, b : b + 1], scalar2=None,
                    op0=mybir.AluOpType.mult,
                )
            nc.sync.dma_start(
                out=y2loc[:].rearrange("(b p) c -> p b c", p=128),
                in_=y2sb[:].rearrange("p (b c) -> p b c", b=NB),
            )
            # ---------------- exchange ----------------
            nc.gpsimd.collective_compute(
                "AllGather", mybir.AluOpType.bypass,
                replica_groups=[list(range(NC))],
                ins=[y2loc[:].opt()], outs=[y2full[:].opt()],
            )
            # ---------------- layer 2 ----------------
            T2 = cpool.tile([128, SLOTS], dt.bfloat16)
            for b in range(NB):
                agg = aggregate(b, K2, offs2_map, idx2_sb, rowloc2_sb, src2)
                nc.vector.tensor_scalar(
                    out=T2[:, b * P : (b + 1) * P], in0=agg[:],
                    scalar1=degO_sb[:, b : b + 1], scalar2=None,
                    op0=mybir.AluOpType.mult,
                )
            TT2 = cpool.tile([128, SLOTS], dt.bfloat16)
            for b in range(NB):
                trp = tr_pool.tile([128, 128], dt.bfloat16, tag="tr")
                nc.tensor.transpose(trp[:], T2[:, b * P : (b + 1) * P], ident_sb[:])
                nc.scalar.copy(TT2[:, b * P : (b + 1) * P], trp[:])
            OUT = cpool.tile([C2, SLOTS], dt.float32)
            for j in range(0, SLOTS, NPROJ):
                n = min(NPROJ, SLOTS - j)
                pp = proj_pool.tile([C2, NPROJ], dt.float32, tag="proj")
                nc.tensor.matmul(pp[:, 0:n], lhsT=w2_sb[:], rhs=TT2[:, j : j + n],
                                 start=True, stop=True)
                nc.scalar.activation(
                    OUT[:, j : j + n], pp[:, 0:n],
                    mybir.ActivationFunctionType.Identity, bias=b2_sb[:, 0:1],
                )
            nc.sync.dma_start(out=out_d[:], in_=OUT[:])
    nc.compile()
    return nc


# ----------------------------------------------------------------------
# Entry point
# ----------------------------------------------------------------------

def kernel(x, deg_inv_sqrt, w1, b1, w2, b2, edge_row, edge_col, num_owned):
    from concourse import bass_utils

    x = np.asarray(x, np.float32)
    deg = np.asarray(deg_inv_sqrt, np.float32)
    sched = _build_schedule(np.asarray(edge_row), np.asarray(edge_col), deg)

    key = (
        sched["K1"].tobytes(), sched["K2"].tobytes(),
    )
    if key not in _PROGRAM_CACHE:
        _PROGRAM_CACHE[key] = _build_program(
            sched["K1"], sched["offs1"], sched["K2"], sched["offs2"]
        )
    nc = _PROGRAM_CACHE[key]

    table1 = np.zeros((V1, C), BF16)
    table1[:N_LOCAL] = (x * deg[:, None]).astype(BF16)
    iota_np = np.tile(np.arange(128, dtype=BF16)[None, :], (128, 1))
    ident_np = np.eye(128, dtype=BF16)
    w1_b = np.asarray(w1, np.float32).astype(BF16)
    w2_b = np.asarray(w2, np.float32).astype(BF16)
    b1_c = np.asarray(b1, np.float32).reshape(C, 1)
    b2_c = np.asarray(b2, np.float32).reshape(C2, 1)

    in_maps = []
    for k in range(NC):
        in_maps.append({
            "table1": table1,
            "idx1": sched["idx16_1"][k],
            "rowloc1": sched["rowloc1"][k],
            "idx2": sched["idx16_2"][k],
            "rowloc2": sched["rowloc2"][k],
            "degO": sched["degO"][k],
            "w1": w1_b, "w2": w2_b, "b1": b1_c, "b2": b2_c,
            "ident": ident_np, "iota": iota_np,
        })
    res = bass_utils.run_bass_kernel_spmd(nc, in_maps, core_ids=list(range(NC)))

    out = np.zeros((N_OWN, C2), np.float32)
    for k in range(NC):
        got = res.results[k]["outT"]  # [C2, SLOTS]
        rows = sched["row_of_slot"][k]
        valid = rows >= 0
        out[rows[valid]] = got[:, valid].T
    return out


# revision 10
# speedup vs baseline: 1.6698x; 1.0277x over previous
"""Distributed 2-layer GCN on 8 NeuronCores (Trainium2, Bass/Tile).

Strategy (graph-partition parallelism):
  - Rows (owned nodes) are degree-sorted and dealt round-robin to the 8
    cores in 128-row blocks so every core gets an identical static
    schedule (SPMD: one traced program).
  - Both GCN layers are computed "aggregate-first":
        out = ((A @ (x*deg)) * deg) @ W + b
    which is algebraically identical to the reference
    (D^-1/2 A D^-1/2 x W + b) because the row/col scalings and the dense
    projection commute with the sparse aggregation.
  - The sparse aggregation runs as: bulk int16 dma_gather (4 parallel
    SWDGE queues) of 256B node rows from a DRAM table, then a one-hot
    "scatter matmul" on the PE accumulating each 128-edge chunk into the
    block's PSUM tile.  One-hots are built on the DVE via iota==rowid.
  - The gather universe (55040 / 50176 rows) exceeds int16, so each
    block's edges are split into two overlapping 32768-row windows.
  - Layer-2 inputs (y2 = relu(out1)*deg) are exchanged with a DRAM
    AllGather across the 8 cores, then layer 2 repeats the same pipeline
    reading from the gathered table.
"""

import numpy as np
import ml_dtypes

N_LOCAL = 55000
N_OWN = 50000
N_EDGES = 800000
C = 128          # in/hidden channels
C2 = 64          # out channels
NC = 8
P = 128
GROUP = NC * P                    # 1024 rows dealt per block index
NB = (N_OWN + GROUP - 1) // GROUP  # 49 blocks per core
SLOTS = NB * P                    # 6272 row slots per core
V1 = 55040                        # layer-1 gather table rows (padded)
W16 = 32768                       # int16 window width
BASE1 = V1 - W16                  # 22272
V2 = NC * SLOTS                   # 50176 layer-2 table rows
BASE2 = V2 - W16                  # 17408
BF16 = ml_dtypes.bfloat16

_PROGRAM_CACHE = {}


# ----------------------------------------------------------------------
# Host-side schedule construction (pure numpy; edges are inputs)
# ----------------------------------------------------------------------

def _build_schedule(edge_row, edge_col, deg):
    """Returns per-core index/one-hot tensors + static chunk schedule."""
    er = edge_row.astype(np.int64)
    ec = edge_col.astype(np.int64)
    keep = er < N_OWN
    er, ec = er[keep], ec[keep]

    deg_cnt = np.bincount(er, minlength=N_OWN)
    order = np.argsort(-deg_cnt, kind="stable").astype(np.int64)  # rank -> row
    inv_order = np.empty(N_OWN, np.int64)
    inv_order[order] = np.arange(N_OWN)

    # rank -> (core, block, partition);  slot s of rank r:
    #   g = r // 1024, lane = (r % 1024) // 128, p = r % 128
    rank_of = inv_order  # row -> rank
    # per-edge destination
    e_rank = rank_of[er]
    e_g = e_rank // GROUP
    e_lane = (e_rank % GROUP) // P
    e_p = e_rank % P

    # layer-2 source position of a col (only cols < N_OWN)
    def pos2_of(col):
        r = rank_of[col]
        return (r % GROUP) // P * SLOTS + (r // GROUP) * P + (r % P)

    # ---- per (core, block, window) edge lists --------------------------
    # layer 1: idx = col (table1 position), window by col
    # layer 2: idx = pos2(col), dropped if col >= N_OWN
    lists1 = [[[[], []] for _ in range(NB)] for _ in range(NC)]
    lists2 = [[[[], []] for _ in range(NB)] for _ in range(NC)]
    l2_valid = ec < N_OWN
    e_pos2 = np.zeros(len(ec), np.int64)
    e_pos2[l2_valid] = pos2_of(ec[l2_valid])
    for i in range(len(er)):
        k, b, p = e_lane[i], e_g[i], e_p[i]
        c1 = ec[i]
        w1 = 0 if c1 < W16 else 1
        lists1[k][b][w1].append((c1 - (BASE1 if w1 else 0), p))
        if l2_valid[i]:
            c2 = e_pos2[i]
            w2 = 0 if c2 < W16 else 1
            lists2[k][b][w2].append((c2 - (BASE2 if w2 else 0), p))

    def pack(lists):
        # static chunk counts (max over cores)
        K = np.zeros((NB, 2), np.int64)
        for b in range(NB):
            for w in range(2):
                n = max(len(lists[k][b][w]) for k in range(NC))
                K[b, w] = (n + P - 1) // P
            if K[b, 0] + K[b, 1] == 0:
                K[b, 0] = 1  # ensure PSUM init via all-dead chunk
        tot_chunks = int(K.sum())
        tot_idx = tot_chunks * P
        idx16 = np.zeros((NC, 128, tot_idx // 16), np.int16)
        rowloc = np.full((NC, 128, tot_chunks), 128.0, BF16)
        off_chunk = 0
        offs = []
        for b in range(NB):
            for w in range(2):
                kc = int(K[b, w])
                offs.append((b, w, off_chunk, kc))
                if kc == 0:
                    continue
                n_idx = kc * P
                for k in range(NC):
                    lst = lists[k][b][w]
                    loc = np.zeros(n_idx, np.int64)
                    rl = np.full(n_idx, 128.0, np.float32)
                    if lst:
                        a = np.asarray(lst, np.int64)
                        loc[: len(a)] = a[:, 0]
                        rl[: len(a)] = a[:, 1]
                    # idx j -> partition j%128, stripe j//128; chunk c = js [c*128,(c+1)*128)
                    rowloc[k, :, off_chunk : off_chunk + kc] = (
                        rl.reshape(kc, P).T
                    )
                    wrapped = loc.reshape(n_idx // 16, 16).T.astype(np.int16)  # [16, S16]
                    idx16[k, :, off_chunk * 8 : (off_chunk + kc) * 8] = np.tile(
                        wrapped, (8, 1)
                    )
                off_chunk += kc
        return K, idx16, rowloc, offs

    K1, idx16_1, rowloc1, offs1 = pack(lists1)
    K2, idx16_2, rowloc2, offs2 = pack(lists2)

    # per-core owned-row deg (0 for pad slots), [128, NB]
    degO = np.zeros((NC, 128, NB), np.float32)
    row_of_slot = np.full((NC, SLOTS), -1, np.int64)
    for k in range(NC):
        for b in range(NB):
            ranks = b * GROUP + k * P + np.arange(P)
            valid = ranks < N_OWN
            rows = np.where(valid, order[np.minimum(ranks, N_OWN - 1)], -1)
            row_of_slot[k, b * P : (b + 1) * P] = rows
            degO[k, valid, b] = deg[rows[valid]]
    return dict(
        K1=K1, idx16_1=idx16_1, rowloc1=rowloc1, offs1=offs1,
        K2=K2, idx16_2=idx16_2, rowloc2=rowloc2, offs2=offs2,
        degO=degO, row_of_slot=row_of_slot, order=order,
    )


# ----------------------------------------------------------------------
# Device program
# ----------------------------------------------------------------------

def _build_program(K1, offs1, K2, offs2):
    import concourse.bass as bass
    import concourse.bacc as bacc
    import concourse.tile as tile
    import concourse.mybir as mybir

    S16_1 = int(K1.sum()) * 8
    NCH1 = int(K1.sum())
    S16_2 = int(K2.sum()) * 8
    NCH2 = int(K2.sum())
    KMAX = int(max(K1.max(), K2.max()))

    nc = bacc.Bacc("TRN2", target_bir_lowering=False, debug=False,
                   num_devices=NC, num_swdge_queues=4)
    dt = mybir.dt
    table1 = nc.dram_tensor("table1", [V1, C], dt.bfloat16, kind="ExternalInput")
    idx1_d = nc.dram_tensor("idx1", [128, S16_1], dt.int16, kind="ExternalInput")
    rowloc1_d = nc.dram_tensor("rowloc1", [128, NCH1], dt.bfloat16, kind="ExternalInput")
    idx2_d = nc.dram_tensor("idx2", [128, S16_2], dt.int16, kind="ExternalInput")
    rowloc2_d = nc.dram_tensor("rowloc2", [128, NCH2], dt.bfloat16, kind="ExternalInput")
    degO_d = nc.dram_tensor("degO", [128, NB], dt.float32, kind="ExternalInput")
    w1_d = nc.dram_tensor("w1", [C, C], dt.bfloat16, kind="ExternalInput")
    w2_d = nc.dram_tensor("w2", [C, C2], dt.bfloat16, kind="ExternalInput")
    b1_d = nc.dram_tensor("b1", [C, 1], dt.float32, kind="ExternalInput")
    b2_d = nc.dram_tensor("b2", [C2, 1], dt.float32, kind="ExternalInput")
    ident_d = nc.dram_tensor("ident", [128, 128], dt.bfloat16, kind="ExternalInput")
    iota_d = nc.dram_tensor("iota", [128, 128], dt.bfloat16, kind="ExternalInput")
    out_d = nc.dram_tensor("outT", [C2, SLOTS], dt.float32, kind="ExternalOutput")

    qrr = [0]

    def next_q():
        q = qrr[0]
        qrr[0] = (q + 1) % 4
        return q

    with tile.TileContext(nc) as tc:
        with (
            tc.tile_pool(name="const", bufs=1) as cpool,
            tc.tile_pool(name="gather", bufs=6) as gpool,
            tc.tile_pool(name="onehot", bufs=8) as opool,
            tc.tile_pool(name="agg", bufs=4, space="PSUM") as agg_pool,
            tc.tile_pool(name="trp", bufs=2, space="PSUM") as tr_pool,
            tc.tile_pool(name="proj", bufs=2, space="PSUM") as proj_pool,
            tc.tile_pool(name="dram", bufs=1, space="DRAM") as dpool,
        ):
            idx1_sb = cpool.tile([128, S16_1], dt.int16)
            nc.sync.dma_start(out=idx1_sb[:], in_=idx1_d[:])
            rowloc1_sb = cpool.tile([128, NCH1], dt.bfloat16)
            nc.sync.dma_start(out=rowloc1_sb[:], in_=rowloc1_d[:])
            idx2_sb = cpool.tile([128, S16_2], dt.int16)
            nc.sync.dma_start(out=idx2_sb[:], in_=idx2_d[:])
            rowloc2_sb = cpool.tile([128, NCH2], dt.bfloat16)
            nc.sync.dma_start(out=rowloc2_sb[:], in_=rowloc2_d[:])
            degO_sb = cpool.tile([128, NB], dt.float32)
            nc.sync.dma_start(out=degO_sb[:], in_=degO_d[:])
            w1_sb = cpool.tile([C, C], dt.bfloat16)
            nc.sync.dma_start(out=w1_sb[:], in_=w1_d[:])
            w2_sb = cpool.tile([C, C2], dt.bfloat16)
            nc.sync.dma_start(out=w2_sb[:], in_=w2_d[:])
            b1_sb = cpool.tile([C, 1], dt.float32)
            nc.sync.dma_start(out=b1_sb[:], in_=b1_d[:])
            b2_sb = cpool.tile([C2, 1], dt.float32)
            nc.sync.dma_start(out=b2_sb[:], in_=b2_d[:])
            ident_sb = cpool.tile([128, 128], dt.bfloat16)
            nc.sync.dma_start(out=ident_sb[:], in_=ident_d[:])
            iota_sb = cpool.tile([128, 128], dt.bfloat16)
            nc.sync.dma_start(out=iota_sb[:], in_=iota_d[:])

            y2loc = dpool.tile([SLOTS, C], dt.bfloat16)
            y2full = dpool.tile([V2, C], dt.bfloat16)

            def aggregate(block, Ktab, offs_map, idx_sb, rowloc_sb, srcs):
                """One 128-row block: gathers + one-hot scatter matmuls."""
                agg = agg_pool.tile([128, C], dt.float32, tag="agg")
                chunks = [(w, off, kc) for (b, w, off, kc) in offs_map[block] if kc > 0]
                total = sum(kc for (_, _, kc) in chunks)
                done = 0
                for (w, off, kc) in chunks:
                    g = gpool.tile([128, KMAX, C], dt.bfloat16, tag="g")
                    n_idx = kc * P
                    nc.gpsimd.dma_gather(
                        out_ap=g[:, 0:kc, :],
                        in_ap=srcs[w],
                        idxs_ap=idx_sb[:, off * 8 : (off + kc) * 8],
                        num_idxs=n_idx, num_idxs_reg=n_idx,
                        elem_size=C, queue_num=next_q(),
                        single_packet=(n_idx <= 1024),
                    )
                    for c in range(kc):
                        S = opool.tile([128, 128], dt.bfloat16, tag="S")
                        nc.vector.tensor_tensor(
                            out=S[:], in0=iota_sb[:],
                            in1=rowloc_sb[:, off + c : off + c + 1].to_broadcast([128, 128]),
                            op=mybir.AluOpType.is_equal,
                        )
                        nc.tensor.matmul(
                            agg[:], lhsT=S[:], rhs=g[:, c, :],
                            start=(done == 0), stop=(done == total - 1),
                        )
                        done += 1
                return agg

            # organize offs per block
            offs1_map = [[] for _ in range(NB)]
            for t in offs1:
                offs1_map[t[0]].append(t)
            offs2_map = [[] for _ in range(NB)]
            for t in offs2:
                offs2_map[t[0]].append(t)

            src1 = [table1[0:W16, :], table1[BASE1:, :]]
            src2 = [y2full[0:W16, :], y2full[BASE2:, :]]

            # ---------------- layer 1 ----------------
            T1 = cpool.tile([128, SLOTS], dt.bfloat16)
            for b in range(NB):
                agg = aggregate(b, K1, offs1_map, idx1_sb, rowloc1_sb, src1)
                nc.vector.tensor_scalar(
                    out=T1[:, b * P : (b + 1) * P], in0=agg[:],
                    scalar1=degO_sb[:, b : b + 1], scalar2=None,
                    op0=mybir.AluOpType.mult,
                )
            # transpose blocks: T1 (node-major) -> TT1 (feature-major)
            TT1 = cpool.tile([128, SLOTS], dt.bfloat16)
            for b in range(NB):
                trp = tr_pool.tile([128, 128], dt.bfloat16, tag="tr")
                nc.tensor.transpose(trp[:], T1[:, b * P : (b + 1) * P], ident_sb[:])
                nc.scalar.copy(TT1[:, b * P : (b + 1) * P], trp[:])
            # projection + relu + bias
            X2T = cpool.tile([128, SLOTS], dt.bfloat16)
            NPROJ = 512
            for j in range(0, SLOTS, NPROJ):
                n = min(NPROJ, SLOTS - j)
                pp = proj_pool.tile([128, NPROJ], dt.float32, tag="proj")
                nc.tensor.matmul(pp[:, 0:n], lhsT=w1_sb[:], rhs=TT1[:, j : j + n],
                                 start=True, stop=True)
                nc.scalar.activation(
                    X2T[:, j : j + n], pp[:, 0:n],
                    mybir.ActivationFunctionType.Relu, bias=b1_sb[:, 0:1],
                )
            # back-transpose + *deg -> y2 (node-major, bf16) -> DRAM
            y2sb = cpool.tile([128, SLOTS], dt.bfloat16)
            for b in range(NB):
                trp = tr_pool.tile([128, 128], dt.bfloat16, tag="tr")
                nc.tensor.transpose(trp[:], X2T[:, b * P : (b + 1) * P], ident_sb[:])
                nc.vector.tensor_scalar(
                    out=y2sb[:, b * P : (b + 1) * P], in0=trp[:],
                    scalar1=degO_sb[:, b : b + 1], scalar2=None,
                    op0=mybir.AluOpType.mult,
                )
            nc.sync.dma_start(
                out=y2loc[:].rearrange("(b p) c -> p b c", p=128),
                in_=y2sb[:].rearrange("p (b c) -> p b c", b=NB),
            )
            # ---------------- exchange ----------------
            nc.gpsimd.collective_compute(
                "AllGather", mybir.AluOpType.bypass,
                replica_groups=[list(range(NC))],
                ins=[y2loc[:].opt()], outs=[y2full[:].opt()],
            )
            # ---------------- layer 2 ----------------
            T2 = cpool.tile([128, SLOTS], dt.bfloat16)
            for b in range(NB):
                agg = aggregate(b, K2, offs2_map, idx2_sb, rowloc2_sb, src2)
                nc.vector.tensor_scalar(
                    out=T2[:, b * P : (b + 1) * P], in0=agg[:],
                    scalar1=degO_sb[:, b : b + 1], scalar2=None,
                    op0=mybir.AluOpType.mult,
                )
            TT2 = cpool.tile([128, SLOTS], dt.bfloat16)
            for b in range(NB):
                trp = tr_pool.tile([128, 128], dt.bfloat16, tag="tr")
                nc.tensor.transpose(trp[:], T2[:, b * P : (b + 1) * P], ident_sb[:])
                nc.scalar.copy(TT2[:, b * P : (b + 1) * P], trp[:])
            OUT = cpool.tile([C2, SLOTS], dt.float32)
            for j in range(0, SLOTS, NPROJ):
                n = min(NPROJ, SLOTS - j)
                pp = proj_pool.tile([C2, NPROJ], dt.float32, tag="proj")
                nc.tensor.matmul(pp[:, 0:n], lhsT=w2_sb[:], rhs=TT2[:, j : j + n],
                                 start=True, stop=True)
                nc.scalar.activation(
                    OUT[:, j : j + n], pp[:, 0:n],
                    mybir.ActivationFunctionType.Identity, bias=b2_sb[:, 0:1],
                )
            nc.sync.dma_start(out=out_d[:], in_=OUT[:])
    nc.compile()
    return nc


# ----------------------------------------------------------------------
# Entry point
# ----------------------------------------------------------------------

def kernel(x, deg_inv_sqrt, w1, b1, w2, b2, edge_row, edge_col, num_owned):
    from concourse import bass_utils

    x = np.asarray(x, np.float32)
    deg = np.asarray(deg_inv_sqrt, np.float32)
    sched = _build_schedule(np.asarray(edge_row), np.asarray(edge_col), deg)

    key = (
        sched["K1"].tobytes(), sched["K2"].tobytes(),
    )
    if key not in _PROGRAM_CACHE:
        _PROGRAM_CACHE[key] = _build_program(
            sched["K1"], sched["offs1"], sched["K2"], sched["offs2"]
        )
    nc = _PROGRAM_CACHE[key]

    table1 = np.zeros((V1, C), BF16)
    table1[:N_LOCAL] = (x * deg[:, None]).astype(BF16)
    iota_np = np.tile(np.arange(128, dtype=BF16)[None, :], (128, 1))
    ident_np = np.eye(128, dtype=BF16)
    w1_b = np.asarray(w1, np.float32).astype(BF16)
    w2_b = np.asarray(w2, np.float32).astype(BF16)
    b1_c = np.asarray(b1, np.float32).reshape(C, 1)
    b2_c = np.asarray(b2, np.float32).reshape(C2, 1)

    in_maps = []
    for k in range(NC):
        in_maps.append({
            "table1": table1,
            "idx1": sched["idx16_1"][k],
            "rowloc1": sched["rowloc1"][k],
            "idx2": sched["idx16_2"][k],
            "rowloc2": sched["rowloc2"][k],
            "degO": sched["degO"][k],
            "w1": w1_b, "w2": w2_b, "b1": b1_c, "b2": b2_c,
            "ident": ident_np, "iota": iota_np,
        })
    res = bass_utils.run_bass_kernel_spmd(nc, in_maps, core_ids=list(range(NC)))

    out = np.zeros((N_OWN, C2), np.float32)
    for k in range(NC):
        got = res.results[k]["outT"]  # [C2, SLOTS]
        rows = sched["row_of_slot"][k]
        valid = rows >= 0
        out[rows[valid]] = got[:, valid].T
    return out
